# revision 9
# baseline (speedup 1.0000x reference)
"""Trainium2 Bass kernel for per-sample covariance pooling + fc + L2 norm.

Reference computation (per sample b of B=32):
    xc  = x[b] - mean(x[b], axis=0)            # x[b]: [N=20000, D=64]
    cov = xc.T @ xc / (N-1)                    # [64, 64]
    out = normalize(cov.flatten() @ W + b)     # [256]

Kernel formulation (scale/norm invariant):
    G = x.T @ x, s = sum(x, axis=0)            # one PE pass over x
    cov = (G - s s^T / N) / (N-1)
    out = normalize(cov.flatten() @ W + b)

Sharding: data-parallel over batch, 4 samples per core on 8 cores; W
and bias replicated. x is host-packed to fp8 e4m3 (end-to-end rel err
~2.3e-3 vs the 2e-2 gate). Two samples ride side by side per
partition row: chunk layout [x_a(64) | x_b(64) | ones(1)], so the
Gram matmul has a 128-column stationary operand (Fast Weight Load)
and one matmul per 128 rows yields both samples' G blocks plus both
column sums (from the ones column) in a [128, 129] psum.

The rank-1 mean correction is folded into the same psum accumulation:
after the Gram stream, s is transposed to a row (32x32 DVE block
transpose), scaled by -1/N, and eight tiny outer-product matmuls
accumulate -s s^T/N into the two G blocks (32x32 sub-blocks at
partition bases 0/32/64/96). feat = pg * 1/(N-1) then needs only a
plain tensor_scalar per parity half -- no separate R psum, no
SBUF-staged R, no stitch copies.

DMA schedule: pair-0 starts with small tiles (8/16/20 chunks) so the
first Gram matmul can start ~2 us earlier; tiles alternate between
the two HWDGE rings in consumption order (each ring sustains only
~170 GB/s; together ~340). The bias rides ring1 *after* the x stream
(it previously delayed ring1's first x tile by ~1.5 us) and W slices
alternate rings after all x. Warmup dummy matmuls lift the HAM clock
throttle before the first tile lands; a few bridge dummies keep the
clock up across the post-Gram DVE window.
"""

import sys

import numpy as np
import ml_dtypes

for _p in ("/opt/trn_rl_repo",):
    if _p not in sys.path:
        sys.path.append(_p)

# Problem shapes (hardcoded per contract).
B, N, D, OUT = 32, 20000, 64, 256
NCORES = 8
BPC = B // NCORES            # samples per core
NPAIR = BPC // 2             # sample pairs per core
P = 128                      # SBUF partitions / matmul contraction tile
NCH = (N + P - 1) // P       # 157 contraction chunks of 128 rows
NPAD = NCH * P               # 20096 rows after zero padding
FB = 2 * D + 1               # bytes per partition per chunk (pair + ones)
KC = (D * D) // P            # 32 fc contraction chunks
WSLICES = 8                  # W DMA slices (each covers 4 fc chunks)
# x DMA schedule per pair: (chunk offset, chunks). Pair 0 leads with
# small tiles so the Gram stream starts as soon as possible; pair 1
# has slack and uses big tiles.
DMA_TILES_P0 = [(0, 8), (8, 16), (24, 20), (44, 28), (72, 28), (100, 28), (128, 29)]
DMA_TILES_P1 = [(0, 28), (28, 28), (56, 28), (84, 28), (112, 28), (140, 17)]
TILES = [DMA_TILES_P0, DMA_TILES_P1]
# ring per tile, in consumption order (alternate so delivery uses both
# rings' bandwidth for the pair currently being consumed)
RINGS_P0 = [0, 1, 0, 1, 0, 1, 0]
RINGS_P1 = [1, 0, 1, 0, 1, 0]
TRINGS = [RINGS_P0, RINGS_P1]
WRINGS = [1, 0, 1, 0, 1, 0, 1, 0]

_CACHE = {}


def _split_drain_and_barrier(self, tick_clock, wait_clock):
    """Replacement for TileContext._drain_and_barrier emitting one drain per
    sem wait: this walrus vintage rejects >1 sync-wait per instruction."""
    import bass_rust
    import concourse.mybir as mybir

    drain_bi = self.nc.sync.drain()
    inst = drain_bi.ins
    wait_clock.add_sem_waits(
        drain_bi.ins, bass_rust.ScopedClock({None: tick_clock.global_clock})
    )
    waits = list(inst.sync_info.on_wait) if inst.sync_info else []
    if len(waits) > 1:
        # one pure sem-wait NoOp per extra wait (cheaper than extra drains)
        inst.sync_info = mybir.SyncInfo(on_wait=waits[:1], on_update=[])
        for w in waits[1:]:
            nop = mybir.InstNoOp(
                name=f"tailwait-{w.ant_name}",
                engine=mybir.EngineType.SP,
                sync_info=mybir.SyncInfo(on_wait=[w], on_update=[]),
                bass_nofuse=True,
            )
            self.nc.sync.add_instruction(nop)

    self.nc.all_engine_barrier()
    assert self.sems is not None
    popped = self.nc._tile_sem_poison_stack.pop()
    assert popped is self._sem_poison
    self.nc.clear_and_free_semaphores(list(self.sems.allocated().values()))
    self.nc.all_engine_barrier()


def _build_nc():
    import types

    import concourse.bass as bass
    import concourse.mybir as mybir
    from concourse.tile import TileContext

    dt = mybir.dt
    nc = bass.Bass()

    xin = nc.dram_tensor(
        "xin", [NPAIR, NCH * FB * P], dt.float8e4, kind="ExternalInput"
    )
    win = nc.dram_tensor("win", [P, KC * OUT], dt.float16, kind="ExternalInput")
    # cols 0:OUT: bias; cols OUT:OUT+BPC: ones (same row -- matmul
    # operands must start at a 32-multiple partition)
    bin_ = nc.dram_tensor("bin", [1, OUT + BPC], dt.float32, kind="ExternalInput")
    yout = nc.dram_tensor("yout", [BPC, OUT], dt.float32, kind="ExternalOutput")

    # Walrus single-sync-wait discipline (see _split_drain_and_barrier):
    #  - x tiles get one pool slot per DMA (no slot reuse -> DMAs need 0
    #    waits), per-pair psum G tiles are not reused
    #  - cross-engine joins funnel through single producers so each
    #    consumer carries at most one sem wait
    #  - PE "observes" each W slice's DMA lane via a dummy matmul right
    #    before the first fc matmul that reads the slice.
    tc = TileContext(nc)
    tc._drain_and_barrier = types.MethodType(_split_drain_and_barrier, tc)
    with tc:
        with (
            tc.tile_pool(name="const", bufs=1) as cpool,
            tc.tile_pool(name="xp", bufs=len(DMA_TILES_P0) + len(DMA_TILES_P1)) as xpool,
            tc.tile_pool(name="small", bufs=4) as spool,
            tc.tile_pool(name="featp", bufs=1) as fpool,
            tc.tile_pool(name="gpsum", bufs=NPAIR, space="PSUM") as gpool,
            tc.tile_pool(name="opsum", bufs=1, space="PSUM") as opool,
        ):
            w_sb = cpool.tile([P, KC * OUT], dt.float16)
            bias_sb = cpool.tile([1, OUT + BPC], dt.float32)

            ring = [nc.sync, nc.scalar]

            # feat_sb[p, c, bb] = flattened cov for sample bb, fc-chunk
            # layout: element k = c*128 + p of cov.flatten(); chunk c stacks
            # cov[:, 2c] on partitions 0:64 and cov[:, 2c+1] on 64:128.
            feat_sb = fpool.tile([P, KC, BPC], dt.float16)

            po = opool.tile([BPC, OUT], dt.float32)
            pdum = opool.tile([1, 512], dt.float32, tag="pdum")

            # s column scratch (only col 0 written; the 32x32 block
            # transpose routes in-col j to out-row j, so the junk in cols
            # 1:32 lands only on output rows we never read).
            s128 = cpool.tile([P, 32], dt.float32, tag="s128")

            # Pre-warm the PE clock gate (HAM) with dummy matmuls on a memset
            # tile while the first x tile is still in flight: the gate needs
            # ~3.4 us of sustained activity to lift the cold throttle.
            dumsrc = cpool.tile([P, 512], dt.float8e4)
            nc.vector.memset(dumsrc[:], 0.5)

            def dummy_mm(n, cols=256):
                for _ in range(n):
                    nc.tensor.matmul(
                        pdum[:, 0:cols], lhsT=dumsrc[:, 0:1],
                        rhs=dumsrc[:, 0:cols], start=True, stop=True,
                    )

            # x DMAs are emitted per pair, adjacent to their consuming
            # Gram matmuls (the Tile scheduler kept per-ring DMA order
            # stable in this structure; a flat up-front emission got
            # reordered); tile delivery is serial per ring (~170 GB/s).
            xts = [[], []]

            def xdma(q):
                for ti, (i0, nblk) in enumerate(TILES[q]):
                    xt = xpool.tile([P, nblk * FB], dt.float8e4, tag="xt")
                    xts[q].append(xt)
                    ring[TRINGS[q][ti]].dma_start(
                        out=xt[:],
                        in_=xin[q, i0 * FB * P : (i0 + nblk) * FB * P].rearrange(
                            "(p f) -> p f", p=P
                        ),
                    )

            # bias via SWDGE: GpSimd is otherwise idle until the
            # output DMA, and this keeps the 1KB transfer (plus the ~1.4us
            # first-DMA ring-init) off the x-carrying HWDGE rings.
            nc.gpsimd.dma_start(out=bias_sb[:], in_=bin_[:])

            xdma(0)
            dummy_mm(7)

            pgs = []

            def gram(q):
                # pg[0:64, 0:64] = G of sample 2q, pg[64:128, 64:128] = G of
                # sample 2q+1, pg[64q', 128] = s of each. The off-diagonal
                # blocks are cross-sample junk (finite, unread).
                pg = gpool.tile([P, FB], dt.float32, tag="pg")
                pgs.append(pg)
                for ti, (i0, nblk) in enumerate(TILES[q]):
                    xt = xts[q][ti]
                    for j in range(nblk):
                        nc.tensor.matmul(
                            pg[:],
                            lhsT=xt[:, j * FB : j * FB + P],
                            rhs=xt[:, j * FB : (j + 1) * FB],
                            start=(i0 + j == 0),
                            stop=False,
                        )

            def schain(q):
                # s (psum col 128, both samples stacked) -> SBUF column ->
                # 32x32 block transpose puts s[32k:32k+32] into row 32k ->
                # stitch the four 32-wide pieces into a [1, 128] row (plain
                # and scaled by -1/N) so the outer-product matmul sees both
                # operands at partition 0.
                pg = pgs[q]
                nc.vector.tensor_copy(s128[:, 0:1], pg[:, 2 * D : FB])
                sT = spool.tile([P, 32], dt.float32, tag="sT")
                nc.vector.transpose(sT[:], s128[:])
                spos = spool.tile([1, P], dt.float32, tag="spos")
                sscl = spool.tile([1, P], dt.float32, tag="sscl")
                for h in range(4):
                    nc.vector.tensor_copy(
                        spos[0:1, 32 * h : 32 * h + 32], sT[32 * h : 32 * h + 1, :]
                    )
                    nc.vector.tensor_scalar_mul(
                        sscl[0:1, 32 * h : 32 * h + 32],
                        sT[32 * h : 32 * h + 1, :], -1.0 / N,
                    )
                return spos, sscl

            def rank1(q, spos, sscl):
                # accumulate -s s^T / N into each sample's G block.
                pg = pgs[q]
                for bb in range(2):
                    base = D * bb
                    nc.tensor.matmul(
                        pg[base : base + D, base : base + D],
                        lhsT=sscl[0:1, base : base + D],
                        rhs=spos[0:1, base : base + D],
                        start=False,
                        stop=(bb == 1),
                        skip_group_check=True,
                    )

            def feats(q, halves):
                # feat = pg * 1/(N-1)  (= cov), cast to fp16. Two parity
                # halves per sample; optionally split the chunk range so the
                # fc can start while the second half is still on DVE.
                pg = pgs[q]
                HC = KC // halves
                for h in range(halves):
                    for bb in range(2):
                        base = D * bb
                        ge = pg[base : base + D, base : base + D].rearrange(
                            "p (c two) -> p c two", two=2
                        )
                        for par in range(2):
                            nc.vector.tensor_scalar_mul(
                                feat_sb[par * D : par * D + D,
                                        h * HC : (h + 1) * HC, 2 * q + bb],
                                ge[:, h * HC : (h + 1) * HC, par],
                                1.0 / (N - 1.0),
                            )

            # pair 0 Gram stream, then its s-chain on DVE while pair 1's
            # stream begins; pair 0's rank-1 matmuls slot in after pair 1's
            # first tile so they never stall the PE (their sT input is long
            # ready by then).
            gram(0)
            spos0, sscl0 = schain(0)

            xdma(1)
            WSL = KC * OUT // WSLICES
            for c in range(WSLICES):
                ring[WRINGS[c]].dma_start(
                    out=w_sb[:, c * WSL : (c + 1) * WSL],
                    in_=win[:, c * WSL : (c + 1) * WSL],
                )

            # pair 1 gram: first tile, then pair-0 rank1 + feats, then rest
            pg1 = gpool.tile([P, FB], dt.float32, tag="pg")
            pgs.append(pg1)
            i0, nblk = TILES[1][0]
            xt = xts[1][0]
            for j in range(nblk):
                nc.tensor.matmul(
                    pg1[:], lhsT=xt[:, j * FB : j * FB + P],
                    rhs=xt[:, j * FB : (j + 1) * FB],
                    start=(i0 + j == 0), stop=False,
                )
            rank1(0, spos0, sscl0)
            feats(0, halves=1)
            for ti in range(1, len(TILES[1])):
                i0, nblk = TILES[1][ti]
                xt = xts[1][ti]
                for j in range(nblk):
                    nc.tensor.matmul(
                        pg1[:], lhsT=xt[:, j * FB : j * FB + P],
                        rhs=xt[:, j * FB : (j + 1) * FB],
                        start=False, stop=False,
                    )
            # fix pgs bookkeeping: pgs[1] is pg1
            spos1, sscl1 = schain(1)
            dummy_mm(4)          # keep PE active while DVE runs the s-chain
            rank1(1, spos1, sscl1)
            dummy_mm(6)          # bridge the feat window (HAM re-throttle)
            feats(1, halves=2)

            # Open the fc accumulation with the bias row: po = 1 * bias'.
            nc.tensor.matmul(
                po[:], lhsT=bias_sb[0:1, OUT : OUT + BPC], rhs=bias_sb[0:1, 0:OUT],
                start=True, stop=False,
            )
            # fc: out[bb, o] = bias'[o] + sum_k feat[k, bb] * W[k, o].
            # Before the first chunk of each W slice, a 1x1 dummy matmul
            # observes that slice's DMA lane so the fc matmul itself only
            # needs its feat (DVE) wait.
            CPS = KC // WSLICES
            for c in range(KC):
                if c % CPS == 0:
                    sl = c // CPS
                    nc.tensor.matmul(
                        pdum[0:1, 0:1],
                        lhsT=w_sb[0:1, sl * WSL : sl * WSL + 1],
                        rhs=w_sb[0:1, sl * WSL : sl * WSL + 1],
                        start=True, stop=True,
                    )
                nc.tensor.matmul(
                    po[:],
                    lhsT=feat_sb[:, c, :],
                    rhs=w_sb[:, c * OUT : (c + 1) * OUT],
                    start=False,
                    stop=(c == KC - 1),
                )

            # L2 normalize rows: out = po / sqrt(sum(po^2)). ACT Square with
            # row-sum accumulator (a DVE square would need two PSUM reads),
            # ACT sqrt, DVE reciprocal, one DVE scale.
            sq = spool.tile([BPC, OUT], dt.float32, tag="sq")
            ss = spool.tile([BPC, 1], dt.float32, tag="ss")
            nc.scalar.activation(
                sq[:], po[:], mybir.ActivationFunctionType.Square, accum_out=ss[:]
            )
            nrm = spool.tile([BPC, 1], dt.float32, tag="nrm")
            nc.scalar.activation(nrm[:], ss[:], mybir.ActivationFunctionType.Sqrt)
            inv = spool.tile([BPC, 1], dt.float32, tag="inv")
            nc.vector.reciprocal(inv[:], nrm[:])
            out_sb = spool.tile([BPC, OUT], dt.float32, tag="osb")
            nc.vector.tensor_scalar_mul(out_sb[:], po[:], inv[:])
            # SWDGE: an HWDGE yout DMA would need a DMAHW lane-reuse wait on
            # top of its DVE data wait (2 waits > walrus limit).
            nc.gpsimd.dma_start(out=yout[:], in_=out_sb[:])

    return nc


def _get_nc():
    if "nc" not in _CACHE:
        _CACHE["nc"] = _build_nc()
    return _CACHE["nc"]


def _pack_inputs(x, W, b):
    x = np.asarray(x, dtype=np.float32)
    W = np.asarray(W, dtype=np.float32)
    b = np.asarray(b, dtype=np.float32)

    xpad = np.zeros((B, NPAD, D), dtype=ml_dtypes.float8_e4m3)
    xpad[:, :N, :] = x.astype(ml_dtypes.float8_e4m3)
    # Pair samples (2q, 2q+1); chunk i, partition p holds row i*128+p of
    # both samples plus a shared ones byte: [x_a(64) | x_b(64) | 1].
    # [B,NPAD,D] -> [B/2, 2, NCH, P, D] -> [B/2, P, NCH, 2, D]
    xq = xpad.reshape(B // 2, 2, NCH, P, D).transpose(0, 3, 2, 1, 4)
    xq = xq.reshape(B // 2, P, NCH, 2 * D)
    ones = np.ones((B // 2, P, NCH, 1), dtype=ml_dtypes.float8_e4m3)
    augT = np.concatenate([xq, ones], axis=3).reshape(B // 2, P, NCH * FB)
    # regroup into DMA tiles: each dma_start reads one contiguous extent.
    # Pair 0 and pair 1 of each core use different tile schedules.
    rows = []
    for gp in range(B // 2):
        tiles = TILES[gp % NPAIR]
        parts = [
            np.ascontiguousarray(augT[gp, :, i0 * FB : (i0 + nblk) * FB]).reshape(-1)
            for (i0, nblk) in tiles
        ]
        rows.append(np.concatenate(parts))
    augT = np.stack(rows)

    wp = np.ascontiguousarray(
        W.reshape(KC, P, OUT).transpose(1, 0, 2)
    ).reshape(P, KC * OUT).astype(np.float16)
    bp = np.concatenate([b, np.ones(BPC, np.float32)]).reshape(1, OUT + BPC)

    return [
        {
            "xin": np.ascontiguousarray(augT[c * NPAIR : (c + 1) * NPAIR]),
            "win": wp,
            "bin": bp,
        }
        for c in range(NCORES)
    ]


def run(x, W, b, trace=False):
    from concourse.bass_utils import run_bass_kernel_spmd

    nc = _get_nc()
    in_maps = _pack_inputs(x, W, b)
    res = run_bass_kernel_spmd(nc, in_maps, list(range(NCORES)), trace=trace)
    out = np.concatenate(
        [res.results[c]["yout"] for c in range(NCORES)], axis=0
    ).astype(np.float32)
    return out, res


def kernel(x, W, b):
    out, _ = run(x, W, b, trace=False)
    return out


# revision 10
# speedup vs baseline: 1.0673x; 1.0673x over previous
"""Trainium2 Bass kernel for per-sample covariance pooling + fc + L2 norm.

Reference computation (per sample b of B=32):
    xc  = x[b] - mean(x[b], axis=0)            # x[b]: [N=20000, D=64]
    cov = xc.T @ xc / (N-1)                    # [64, 64]
    out = normalize(cov.flatten() @ W + b)     # [256]

Kernel formulation (scale/norm invariant):
    G = x.T @ x, s = sum(x, axis=0)            # one PE pass over x
    cov = (G - s s^T / N) / (N-1)
    out = normalize(cov.flatten() @ W + b)

Sharding: data-parallel over batch, 4 samples per core on 8 cores; W
and bias replicated. x is host-packed to fp8 e4m3 (end-to-end rel err
~2.3e-3 vs the 2e-2 gate). Two samples ride side by side per
partition row: chunk layout [x_a(64) | x_b(64) | ones(1)], so the
Gram matmul has a 128-column stationary operand (Fast Weight Load)
and one matmul per 128 rows yields both samples' G blocks plus both
column sums (from the ones column) in a [128, 129] psum.

The rank-1 mean correction is folded into the same psum accumulation:
after the Gram stream, s is transposed to a row (32x32 DVE block
transpose), scaled by -1/N, and eight tiny outer-product matmuls
accumulate -s s^T/N into the two G blocks (32x32 sub-blocks at
partition bases 0/32/64/96). feat = pg * 1/(N-1) then needs only a
plain tensor_scalar per parity half -- no separate R psum, no
SBUF-staged R, no stitch copies.

DMA schedule: pair-0 starts with small tiles (8/16/20 chunks) so the
first Gram matmul can start ~2 us earlier; tiles alternate between
the two HWDGE rings in consumption order (each ring sustains only
~170 GB/s; together ~340). The bias rides ring1 *after* the x stream
(it previously delayed ring1's first x tile by ~1.5 us) and W slices
alternate rings after all x. Warmup dummy matmuls lift the HAM clock
throttle before the first tile lands; a few bridge dummies keep the
clock up across the post-Gram DVE window.
"""

import sys

import numpy as np
import ml_dtypes

for _p in ("/opt/trn_rl_repo",):
    if _p not in sys.path:
        sys.path.append(_p)

# Problem shapes (hardcoded per contract).
B, N, D, OUT = 32, 20000, 64, 256
NCORES = 8
BPC = B // NCORES            # samples per core
NPAIR = BPC // 2             # sample pairs per core
P = 128                      # SBUF partitions / matmul contraction tile
NCH = (N + P - 1) // P       # 157 contraction chunks of 128 rows
NPAD = NCH * P               # 20096 rows after zero padding
FB = 2 * D + 1               # bytes per partition per chunk (pair + ones)
KC = (D * D) // P            # 32 fc contraction chunks
WSLICES = 8                  # W DMA slices (each covers 4 fc chunks)
# x DMA schedule per pair: (chunk offset, chunks). Pair 0 leads with
# small tiles so the Gram stream starts as soon as possible; pair 1
# has slack and uses big tiles.
def _tiles(sizes):
    out, off = [], 0
    for n in sizes:
        out.append((off, n)); off += n
    assert off == NCH
    return out

DMA_TILES_P0 = _tiles([6, 10, 16, 24, 28, 28, 22, 23])
DMA_TILES_P1 = _tiles([28, 28, 28, 28, 28, 17])
TILES = [DMA_TILES_P0, DMA_TILES_P1]
# ring per tile, in consumption order (alternate so delivery uses both
# rings' bandwidth for the pair currently being consumed)
RINGS_P0 = [0, 1, 0, 1, 0, 1, 0, 1]
RINGS_P1 = [0, 1, 0, 1, 0, 1]
TRINGS = [RINGS_P0, RINGS_P1]
WRINGS = [1, 0, 1, 0, 1, 0, 1, 0]

_CACHE = {}


def _split_drain_and_barrier(self, tick_clock, wait_clock):
    """Replacement for TileContext._drain_and_barrier emitting one drain per
    sem wait: this walrus vintage rejects >1 sync-wait per instruction."""
    import bass_rust
    import concourse.mybir as mybir

    drain_bi = self.nc.sync.drain()
    inst = drain_bi.ins
    wait_clock.add_sem_waits(
        drain_bi.ins, bass_rust.ScopedClock({None: tick_clock.global_clock})
    )
    waits = list(inst.sync_info.on_wait) if inst.sync_info else []
    if len(waits) > 1:
        # one pure sem-wait NoOp per extra wait (cheaper than extra drains)
        inst.sync_info = mybir.SyncInfo(on_wait=waits[:1], on_update=[])
        for w in waits[1:]:
            nop = mybir.InstNoOp(
                name=f"tailwait-{w.ant_name}",
                engine=mybir.EngineType.SP,
                sync_info=mybir.SyncInfo(on_wait=[w], on_update=[]),
                bass_nofuse=True,
            )
            self.nc.sync.add_instruction(nop)

    self.nc.all_engine_barrier()
    assert self.sems is not None
    popped = self.nc._tile_sem_poison_stack.pop()
    assert popped is self._sem_poison
    self.nc.clear_and_free_semaphores(list(self.sems.allocated().values()))
    self.nc.all_engine_barrier()


def _build_nc():
    import types

    import concourse.bass as bass
    import concourse.mybir as mybir
    from concourse.tile import TileContext
    from bass_rust import add_dep_helper

    dt = mybir.dt
    nc = bass.Bass()

    xin = nc.dram_tensor(
        "xin", [NPAIR, NCH * FB * P], dt.float8e4, kind="ExternalInput"
    )
    win = nc.dram_tensor("win", [P, KC * OUT], dt.float16, kind="ExternalInput")
    # cols 0:OUT: bias; cols OUT:OUT+BPC: ones (same row -- matmul
    # operands must start at a 32-multiple partition)
    bin_ = nc.dram_tensor("bin", [1, OUT + BPC], dt.float32, kind="ExternalInput")
    yout = nc.dram_tensor("yout", [BPC, OUT], dt.float32, kind="ExternalOutput")

    # Walrus single-sync-wait discipline (see _split_drain_and_barrier):
    #  - x tiles get one pool slot per DMA (no slot reuse -> DMAs need 0
    #    waits), per-pair psum G tiles are not reused
    #  - cross-engine joins funnel through single producers so each
    #    consumer carries at most one sem wait
    #  - PE "observes" each W slice's DMA lane via a dummy matmul right
    #    before the first fc matmul that reads the slice.
    tc = TileContext(nc)
    tc._drain_and_barrier = types.MethodType(_split_drain_and_barrier, tc)
    with tc:
        with (
            tc.tile_pool(name="const", bufs=1) as cpool,
            tc.tile_pool(name="xp", bufs=len(DMA_TILES_P0) + len(DMA_TILES_P1)) as xpool,
            tc.tile_pool(name="small", bufs=4) as spool,
            tc.tile_pool(name="featp", bufs=1) as fpool,
            tc.tile_pool(name="gpsum", bufs=NPAIR, space="PSUM") as gpool,
            tc.tile_pool(name="opsum", bufs=1, space="PSUM") as opool,
        ):
            w_sb = cpool.tile([P, KC * OUT], dt.float16)
            bias_sb = cpool.tile([1, OUT + BPC], dt.float32)

            ring = [nc.sync, nc.scalar]

            # feat_sb[p, c, bb] = flattened cov for sample bb, fc-chunk
            # layout: element k = c*128 + p of cov.flatten(); chunk c stacks
            # cov[:, 2c] on partitions 0:64 and cov[:, 2c+1] on 64:128.
            feat_sb = fpool.tile([P, KC, BPC], dt.float16)

            po = opool.tile([BPC, OUT], dt.float32)
            pdum = opool.tile([1, 512], dt.float32, tag="pdum")

            # s column scratch (only col 0 written; the 32x32 block
            # transpose routes in-col j to out-row j, so the junk in cols
            # 1:32 lands only on output rows we never read).
            s128 = cpool.tile([P, 32], dt.float32, tag="s128")

            # Pre-warm the PE clock gate (HAM) with dummy matmuls on a memset
            # tile while the first x tile is still in flight: the gate needs
            # ~3.4 us of sustained activity to lift the cold throttle.
            dumsrc = cpool.tile([P, 512], dt.float8e4)
            nc.vector.memset(dumsrc[:], 0.5)

            def dummy_mm(n, cols=256):
                for _ in range(n):
                    nc.tensor.matmul(
                        pdum[:, 0:cols], lhsT=dumsrc[:, 0:1],
                        rhs=dumsrc[:, 0:cols], start=True, stop=True,
                    )

            # All x DMAs up front. The Tile scheduler is free to reorder
            # same-engine instructions (it cost-simulates and picks from a
            # ready heap), which scrambled per-ring DMA order in earlier
            # revisions -- so each ring's DMAs are chained with explicit
            # ordering-only edges (sync=False: no runtime semaphore, the
            # ring executes its issue order anyway). Tile delivery is
            # serial per ring at ~170 GB/s.
            last_dma = [None, None]

            def ring_dma(r, out, in_):
                bi = ring[r].dma_start(out=out, in_=in_)
                if last_dma[r] is not None:
                    add_dep_helper(
                        bi.ins, last_dma[r].ins, sync=False,
                        reason="hwdge ring issue order",
                    )
                last_dma[r] = bi

            xts = [[], []]

            def xdma(q):
                for ti, (i0, nblk) in enumerate(TILES[q]):
                    xt = xpool.tile([P, nblk * FB], dt.float8e4, tag="xt")
                    xts[q].append(xt)
                    ring_dma(
                        TRINGS[q][ti],
                        xt[:],
                        xin[q, i0 * FB * P : (i0 + nblk) * FB * P].rearrange(
                            "(p f) -> p f", p=P
                        ),
                    )

            # bias via SWDGE: GpSimd is otherwise idle until the
            # output DMA, and this keeps the 1KB transfer (plus the ~1.4us
            # first-DMA ring-init) off the x-carrying HWDGE rings.
            nc.gpsimd.dma_start(out=bias_sb[:], in_=bin_[:])

            xdma(0)
            dummy_mm(7)

            pgs = []

            def gram(q, fills=()):
                # pg[0:64, 0:64] = G of sample 2q, pg[64:128, 64:128] = G of
                # sample 2q+1, pg[64q', 128] = s of each. The off-diagonal
                # blocks are cross-sample junk (finite, unread). `fills`
                # inserts HAM-warming dummies after the given tiles so the
                # clock gate sees continuous activity while early tiles
                # trickle in.
                pg = gpool.tile([P, FB], dt.float32, tag="pg")
                pgs.append(pg)
                for ti, (i0, nblk) in enumerate(TILES[q]):
                    xt = xts[q][ti]
                    for j in range(nblk):
                        nc.tensor.matmul(
                            pg[:],
                            lhsT=xt[:, j * FB : j * FB + P],
                            rhs=xt[:, j * FB : (j + 1) * FB],
                            start=(i0 + j == 0),
                            stop=False,
                        )
                    if ti in fills:
                        dummy_mm(1)

            def schain(q):
                # s (psum col 128, both samples stacked) -> SBUF column ->
                # 32x32 block transpose puts s[32k:32k+32] into row 32k ->
                # stitch the four 32-wide pieces into a [1, 128] row (plain
                # and scaled by -1/N) so the outer-product matmul sees both
                # operands at partition 0.
                pg = pgs[q]
                nc.vector.tensor_copy(s128[:, 0:1], pg[:, 2 * D : FB])
                sT = spool.tile([P, 32], dt.float32, tag="sT")
                nc.vector.transpose(sT[:], s128[:])
                spos = spool.tile([1, P], dt.float32, tag="spos")
                sscl = spool.tile([1, P], dt.float32, tag="sscl")
                for h in range(4):
                    nc.vector.tensor_copy(
                        spos[0:1, 32 * h : 32 * h + 32], sT[32 * h : 32 * h + 1, :]
                    )
                    nc.vector.tensor_scalar_mul(
                        sscl[0:1, 32 * h : 32 * h + 32],
                        sT[32 * h : 32 * h + 1, :], -1.0 / N,
                    )
                return spos, sscl

            def rank1(q, spos, sscl):
                # accumulate -s s^T / N into each sample's G block.
                pg = pgs[q]
                for bb in range(2):
                    base = D * bb
                    nc.tensor.matmul(
                        pg[base : base + D, base : base + D],
                        lhsT=sscl[0:1, base : base + D],
                        rhs=spos[0:1, base : base + D],
                        start=False,
                        stop=(bb == 1),
                        skip_group_check=True,
                    )

            def feats(q, halves):
                # feat = pg * 1/(N-1)  (= cov), cast to fp16. Two parity
                # halves per sample; optionally split the chunk range so the
                # fc can start while the second half is still on DVE.
                pg = pgs[q]
                HC = KC // halves
                for h in range(halves):
                    for bb in range(2):
                        base = D * bb
                        ge = pg[base : base + D, base : base + D].rearrange(
                            "p (c two) -> p c two", two=2
                        )
                        for par in range(2):
                            nc.vector.tensor_scalar_mul(
                                feat_sb[par * D : par * D + D,
                                        h * HC : (h + 1) * HC, 2 * q + bb],
                                ge[:, h * HC : (h + 1) * HC, par],
                                1.0 / (N - 1.0),
                            )

            # pair 0 Gram stream, then its s-chain on DVE while pair 1's
            # stream begins; pair 0's rank-1 matmuls slot in after pair 1's
            # first tile so they never stall the PE (their sT input is long
            # ready by then).
            gram(0, fills=(0, 1))
            spos0, sscl0 = schain(0)

            xdma(1)
            WSL = KC * OUT // WSLICES
            for c in range(WSLICES):
                ring_dma(
                    WRINGS[c],
                    w_sb[:, c * WSL : (c + 1) * WSL],
                    win[:, c * WSL : (c + 1) * WSL],
                )

            # pair 1 gram: first tile, then pair-0 rank1 + feats, then rest
            pg1 = gpool.tile([P, FB], dt.float32, tag="pg")
            pgs.append(pg1)
            i0, nblk = TILES[1][0]
            xt = xts[1][0]
            for j in range(nblk):
                nc.tensor.matmul(
                    pg1[:], lhsT=xt[:, j * FB : j * FB + P],
                    rhs=xt[:, j * FB : (j + 1) * FB],
                    start=(i0 + j == 0), stop=False,
                )
            rank1(0, spos0, sscl0)
            feats(0, halves=1)
            for ti in range(1, len(TILES[1])):
                i0, nblk = TILES[1][ti]
                xt = xts[1][ti]
                for j in range(nblk):
                    nc.tensor.matmul(
                        pg1[:], lhsT=xt[:, j * FB : j * FB + P],
                        rhs=xt[:, j * FB : (j + 1) * FB],
                        start=False, stop=False,
                    )
            # fix pgs bookkeeping: pgs[1] is pg1
            spos1, sscl1 = schain(1)
            dummy_mm(4)          # keep PE active while DVE runs the s-chain
            rank1(1, spos1, sscl1)
            dummy_mm(6)          # bridge the feat window (HAM re-throttle)
            feats(1, halves=2)

            # Open the fc accumulation with the bias row: po = 1 * bias'.
            nc.tensor.matmul(
                po[:], lhsT=bias_sb[0:1, OUT : OUT + BPC], rhs=bias_sb[0:1, 0:OUT],
                start=True, stop=False,
            )
            # fc: out[bb, o] = bias'[o] + sum_k feat[k, bb] * W[k, o].
            # Before the first chunk of each W slice, a 1x1 dummy matmul
            # observes that slice's DMA lane so the fc matmul itself only
            # needs its feat (DVE) wait.
            CPS = KC // WSLICES
            for c in range(KC):
                if c % CPS == 0:
                    sl = c // CPS
                    nc.tensor.matmul(
                        pdum[0:1, 0:1],
                        lhsT=w_sb[0:1, sl * WSL : sl * WSL + 1],
                        rhs=w_sb[0:1, sl * WSL : sl * WSL + 1],
                        start=True, stop=True,
                    )
                nc.tensor.matmul(
                    po[:],
                    lhsT=feat_sb[:, c, :],
                    rhs=w_sb[:, c * OUT : (c + 1) * OUT],
                    start=False,
                    stop=(c == KC - 1),
                )

            # L2 normalize rows: out = po / sqrt(sum(po^2)). ACT Square with
            # row-sum accumulator (a DVE square would need two PSUM reads),
            # ACT sqrt, DVE reciprocal, one DVE scale.
            sq = spool.tile([BPC, OUT], dt.float32, tag="sq")
            ss = spool.tile([BPC, 1], dt.float32, tag="ss")
            nc.scalar.activation(
                sq[:], po[:], mybir.ActivationFunctionType.Square, accum_out=ss[:]
            )
            nrm = spool.tile([BPC, 1], dt.float32, tag="nrm")
            nc.scalar.activation(nrm[:], ss[:], mybir.ActivationFunctionType.Sqrt)
            inv = spool.tile([BPC, 1], dt.float32, tag="inv")
            nc.vector.reciprocal(inv[:], nrm[:])
            out_sb = spool.tile([BPC, OUT], dt.float32, tag="osb")
            nc.vector.tensor_scalar_mul(out_sb[:], po[:], inv[:])
            # SWDGE: an HWDGE yout DMA would need a DMAHW lane-reuse wait on
            # top of its DVE data wait (2 waits > walrus limit).
            nc.gpsimd.dma_start(out=yout[:], in_=out_sb[:])

    return nc


def _get_nc():
    if "nc" not in _CACHE:
        _CACHE["nc"] = _build_nc()
    return _CACHE["nc"]


def _pack_inputs(x, W, b):
    x = np.asarray(x, dtype=np.float32)
    W = np.asarray(W, dtype=np.float32)
    b = np.asarray(b, dtype=np.float32)

    xpad = np.zeros((B, NPAD, D), dtype=ml_dtypes.float8_e4m3)
    xpad[:, :N, :] = x.astype(ml_dtypes.float8_e4m3)
    # Pair samples (2q, 2q+1); chunk i, partition p holds row i*128+p of
    # both samples plus a shared ones byte: [x_a(64) | x_b(64) | 1].
    # [B,NPAD,D] -> [B/2, 2, NCH, P, D] -> [B/2, P, NCH, 2, D]
    xq = xpad.reshape(B // 2, 2, NCH, P, D).transpose(0, 3, 2, 1, 4)
    xq = xq.reshape(B // 2, P, NCH, 2 * D)
    ones = np.ones((B // 2, P, NCH, 1), dtype=ml_dtypes.float8_e4m3)
    augT = np.concatenate([xq, ones], axis=3).reshape(B // 2, P, NCH * FB)
    # regroup into DMA tiles: each dma_start reads one contiguous extent.
    # Pair 0 and pair 1 of each core use different tile schedules.
    rows = []
    for gp in range(B // 2):
        tiles = TILES[gp % NPAIR]
        parts = [
            np.ascontiguousarray(augT[gp, :, i0 * FB : (i0 + nblk) * FB]).reshape(-1)
            for (i0, nblk) in tiles
        ]
        rows.append(np.concatenate(parts))
    augT = np.stack(rows)

    wp = np.ascontiguousarray(
        W.reshape(KC, P, OUT).transpose(1, 0, 2)
    ).reshape(P, KC * OUT).astype(np.float16)
    bp = np.concatenate([b, np.ones(BPC, np.float32)]).reshape(1, OUT + BPC)

    return [
        {
            "xin": np.ascontiguousarray(augT[c * NPAIR : (c + 1) * NPAIR]),
            "win": wp,
            "bin": bp,
        }
        for c in range(NCORES)
    ]


def run(x, W, b, trace=False):
    from concourse.bass_utils import run_bass_kernel_spmd

    nc = _get_nc()
    in_maps = _pack_inputs(x, W, b)
    res = run_bass_kernel_spmd(nc, in_maps, list(range(NCORES)), trace=trace)
    out = np.concatenate(
        [res.results[c]["yout"] for c in range(NCORES)], axis=0
    ).astype(np.float32)
    return out, res


def kernel(x, W, b):
    out, _ = run(x, W, b, trace=False)
    return out
